# revision 27
# baseline (speedup 1.0000x reference)
"""Trainium2 Bass kernel for nn_SimpleMLP (segment-mean + 2-layer MLP).

reference:
  sums = segment_sum(x, batch, 4096); cnt = segment_sum(ones, batch, 4096)
  pooled = sums / max(cnt, 1);  out = gelu(pooled @ W1 + b1) @ W2 + b2

v2 design (two-phase block reduction; replaces the per-row one-hot design
that was DVE-bound at ~200us):

  Distribution: `batch` is sorted; core k owns segments [512k, 512k+512).
  The host gathers each core's rows into a packed slab where every segment
  is padded to a whole number of 16-row blocks (pad rows read a zeros row,
  ~3% extra bytes), so each block is segment-pure. x is converted to
  fp8-e4m3 on the host (DMA bytes halve vs fp16; measured output rel err
  stays ~1e-2 < 2e-2 gate).

  Phase 1: fp8 DoubleRow matmuls (Ki=128 x Ko=2 = 256 rows each) against a
  CONSTANT 32-column block indicator produce 16-row block sums directly in
  PSUM ([128 blocks, 256] tiles, 4 col-group slices x 2 indicator phases).
  The moving operand is x, streamed at 2 fp8/partition/cycle (~546 GB/s >
  DMA rate), and LDWEIGHTS is only 64 columns (M=32) so it hides under the
  matmul. The Scalar engine evicts each PSUM tile to SBUF as fp16.

  Phase 2: a small block->segment one-hot (built on DVE from a host
  provided block->segment map, 16x fewer elements than a per-row one-hot)
  is matmul'd with the fp16 block sums, accumulating [128 segs, 256] per
  128-segment window. Host-computed 1/max(cnt,1) turns sums into means.

  Phase 3: replicated tiny MLP (fp32 matmuls, hardware Gelu) on the core's
  512 segments; host concatenates the 8 [512, 256] outputs.
"""
import sys

sys.path.insert(0, "/opt/trn_rl_repo")

from contextlib import ExitStack

import ml_dtypes
import numpy as np

import concourse.bacc as bacc
import concourse.mybir as mybir
import concourse.tile as tile
from concourse import bass_utils

F32 = mybir.dt.float32
F16 = mybir.dt.float16
F8 = mybir.dt.float8e4
FP8NP = ml_dtypes.float8_e4m3

N = 1048576
H = 256
S = 4096
NCORES = 8
SEG_PC = S // NCORES          # 512 segments per core
GROWS = 16                    # rows per block (segment padding granule)
MM_ROWS = 256                 # rows per DoubleRow matmul (Ki=128 * Ko=2)
CH_MMS = 16                   # matmuls per DMA chunk
CH_ROWS = CH_MMS * MM_ROWS    # 4096 rows per 1MB chunk
KQ = 8                        # q-tiles per one-hot DVE instruction

_nc_cache = {}


def _build_nc(nsup, use_gelu=True):
    nq = 4 * nsup             # [64,256] psum tiles (64 blocks each)
    nc = bacc.Bacc("TRN2", target_bir_lowering=False, debug=False,
                   num_devices=NCORES)
    xs_d = nc.dram_tensor("xs", [nsup * 128, CH_MMS * 512], F8,
                          kind="ExternalInput")
    ind_d = nc.dram_tensor("ind", [128, 4, 2, 64], F8, kind="ExternalInput")
    # block->segment map: q-tile 4j+g holds window g's blocks [64j, 64j+64);
    # bbg[b, g, j] = seg_local - 128g of block 64j+b of window g (fp16)
    bbg_d = nc.dram_tensor("bbg", [64, 4, nsup], F16, kind="ExternalInput")
    rcp_d = nc.dram_tensor("rcp", [128, 4], F32, kind="ExternalInput")
    w1_d = nc.dram_tensor("w1", [H, H], F32, kind="ExternalInput")
    b1_d = nc.dram_tensor("b1", [H], F32, kind="ExternalInput")
    w2_d = nc.dram_tensor("w2", [H, H], F32, kind="ExternalInput")
    b2_d = nc.dram_tensor("b2", [H], F32, kind="ExternalInput")
    out_d = nc.dram_tensor("out", [SEG_PC, H], F32, kind="ExternalOutput")

    DR = mybir.MatmulPerfMode.DoubleRow

    with tile.TileContext(nc) as tc, ExitStack() as ctx:
        const = ctx.enter_context(tc.tile_pool(name="const", bufs=1))
        xp = ctx.enter_context(tc.tile_pool(name="xp", bufs=6))
        ohp = ctx.enter_context(tc.tile_pool(name="ohp", bufs=3))
        psw = ctx.enter_context(tc.tile_pool(name="psw", bufs=2, space="PSUM"))
        psx = ctx.enter_context(tc.tile_pool(name="psx", bufs=4, space="PSUM"))
        sb = ctx.enter_context(tc.tile_pool(name="sb", bufs=1))

        # --- constants ---
        # dense per-position iota (values 0..127 repeated over KQ slots) for
        # the block one-hot compare; dense operand rides port 0
        iota_b = const.tile([128, KQ, 128], F16, name="iota_b")
        nc.gpsimd.iota(iota_b[:], pattern=[[0, KQ], [1, 128]], base=0,
                       channel_multiplier=0,
                       allow_small_or_imprecise_dtypes=True)
        pidx = const.tile([128, 1], F32)          # partition index
        nc.gpsimd.iota(pidx[:], pattern=[[0, 1]], base=0, channel_multiplier=1,
                       allow_small_or_imprecise_dtypes=True)
        identcmp = const.tile([128, 128], F32)
        nc.gpsimd.iota(identcmp[:], pattern=[[1, 128]], base=0,
                       channel_multiplier=0,
                       allow_small_or_imprecise_dtypes=True)
        ident = const.tile([128, 128], F32)       # identity for PE transpose
        nc.vector.tensor_scalar(ident[:], identcmp[:], pidx[:], None,
                                op0=mybir.AluOpType.is_equal)

        # --- weights / biases / maps ---
        ind_sb = const.tile([128, 4, 2, 64], F8)
        nc.gpsimd.dma_start(ind_sb[:], ind_d.ap())
        bbg_sb = const.tile([64, 4, nsup], F16)
        nc.gpsimd.dma_start(bbg_sb[:], bbg_d.ap())
        rcp_sb = const.tile([128, 4], F32)
        nc.gpsimd.dma_start(rcp_sb[:], rcp_d.ap())
        w1_sb = const.tile([128, 2, H], F32)
        nc.gpsimd.dma_start(w1_sb[:], w1_d.ap().rearrange("(k p) h -> p k h", p=128))
        w2_sb = const.tile([128, 2, H], F32)
        nc.gpsimd.dma_start(w2_sb[:], w2_d.ap().rearrange("(k p) h -> p k h", p=128))
        b1_sb = const.tile([128, 2], F32)
        nc.gpsimd.dma_start(b1_sb[:], b1_d.ap().rearrange("(m p) -> p m", p=128))
        b2_sb = const.tile([128, 2], F32)
        nc.gpsimd.dma_start(b2_sb[:], b2_d.ap().rearrange("(m p) -> p m", p=128))

        # fp16 16-row block sums, [64 blocks, 256] per q-tile (DoubleRow
        # matmuls must write PSUM at base partition 0, so everything lives
        # on partitions 0..63)
        bsums = const.tile([64, nq, H], F16, name="bsums")

        # --- phase 2 one-hots, pre-generated (depend only on constants) ---
        ohs = {}
        for g in range(4):
            for js in range(0, nsup, KQ):
                je = min(js + KQ, nsup)
                oh = const.tile([64, KQ, 128], F16)
                bcast = (bbg_sb[:, g, js:je]
                         .rearrange("p (q u) -> p q u", u=1)
                         .broadcast_to((64, je - js, 128)))
                nc.vector.tensor_tensor(oh[:, :je - js, :],
                                        iota_b[0:64, :je - js, :], bcast,
                                        op=mybir.AluOpType.is_equal)
                ohs[g, js] = oh

        pooled = sb.tile([128, 4, H], F32)  # window g -> pooled[:, g, :]
        # one live PSUM accumulator per window, fed incrementally as q-tiles
        # are evicted (phase 2 rides the PE slack of the DMA-paced phase 1)
        pg = [psx.tile([128, H], F32, name=f"pg{g}", bufs=1) for g in range(4)]

        def emit_p2(c):
            for u in range(4):
                nc.tensor.matmul(pg[u][:], ohs[u, (c // KQ) * KQ][:, c % KQ, :],
                                 bsums[:, 4 * c + u, :],
                                 start=(c == 0), stop=(c == nsup - 1))

        # --- phase 1: block sums via constant-stationary DoubleRow ---
        hm = CH_MMS // 2
        for c in range(nsup):
            x_sb = xp.tile([128, CH_MMS, 2, H], F8)
            # split each 1MB chunk across the two HWDGE queues so they run
            # concurrently and release consumers at half-chunk granularity
            src = xs_d.ap()[c * 128:(c + 1) * 128, :] \
                .rearrange("p (t k n) -> p t k n", t=CH_MMS, k=2)
            nc.sync.dma_start(x_sb[:, 0:hm, :, :], src[:, 0:hm, :, :])
            nc.scalar.dma_start(x_sb[:, hm:CH_MMS, :, :],
                                src[:, hm:CH_MMS, :, :])
            for u in range(4):          # psum tiles per chunk (4 MMs each)
                pq = psw.tile([64, H], F32, name="pq", tag="pq")
                for a in range(4):      # indicator phase
                    tl = 4 * u + a
                    nc.tensor.matmul(pq[:],
                                     ind_sb[:, a, :, :],
                                     x_sb[:, tl, :, :],
                                     start=(a == 0), stop=(a == 3),
                                     perf_mode=DR)
                q = 4 * c + u
                # split PSUM evictions between ACT and DVE so neither
                # stalls its DMA issue stream
                if u % 2 == 0:
                    nc.scalar.activation(bsums[:, q, :], pq[:],
                                         mybir.ActivationFunctionType.Copy)
                else:
                    nc.vector.tensor_copy(bsums[:, q, :], pq[:])
            # phase-2 accumulation for the PREVIOUS chunk's q-tiles (their
            # evictions are long done, so these never stall the PE queue)
            if c >= 1:
                emit_p2(c - 1)
        emit_p2(nsup - 1)
        for g in range(4):
            nc.vector.tensor_scalar_mul(pooled[:, g, :], pg[g][:],
                                        rcp_sb[:, g:g + 1])

        # --- transpose pooled -> pooledT [128, 2, 512] (h-chunk, seg) ---
        pooledT = sb.tile([128, 2, SEG_PC], F32)
        for g in range(4):
            for j in range(2):
                pt = psx.tile([128, 128], F32, tag="pt", bufs=1)
                nc.tensor.transpose(pt[:], pooled[:, g, j * 128:(j + 1) * 128],
                                    ident[:])
                nc.vector.tensor_copy(pooledT[:, j, g * 128:(g + 1) * 128], pt[:])

        # --- MLP layer 1: hT = gelu(W1.T @ pooledT + b1) ---
        hT = sb.tile([128, 2, SEG_PC], F32)
        for m in range(2):
            ph = psx.tile([128, SEG_PC], F32, tag="ph", bufs=1)
            for k in range(2):
                nc.tensor.matmul(ph[:], w1_sb[:, k, m * 128:(m + 1) * 128],
                                 pooledT[:, k, :], start=(k == 0), stop=(k == 1))
            act = (mybir.ActivationFunctionType.Gelu if use_gelu
                   else mybir.ActivationFunctionType.Identity)
            nc.scalar.activation(hT[:, m, :], ph[:], act,
                                 bias=b1_sb[:, m:m + 1], scale=1.0)

        # --- MLP layer 2: oT = W2.T @ hT + b2 ---
        oT = sb.tile([128, 2, SEG_PC], F32)
        for m in range(2):
            ph = psx.tile([128, SEG_PC], F32, tag="ph", bufs=1)
            for k in range(2):
                nc.tensor.matmul(ph[:], w2_sb[:, k, m * 128:(m + 1) * 128],
                                 hT[:, k, :], start=(k == 0), stop=(k == 1))
            nc.scalar.activation(oT[:, m, :], ph[:],
                                 mybir.ActivationFunctionType.Identity,
                                 bias=b2_sb[:, m:m + 1], scale=1.0)

        # --- transpose back and store ---
        out_sb = sb.tile([128, 4, H], F32)
        for g in range(4):
            for j in range(2):
                pt = psx.tile([128, 128], F32, tag="pt", bufs=1)
                nc.tensor.transpose(pt[:], oT[:, j, g * 128:(g + 1) * 128],
                                    ident[:])
                nc.vector.tensor_copy(out_sb[:, g, j * 128:(j + 1) * 128], pt[:])
        nc.sync.dma_start(out_d.ap().rearrange("(g p) h -> p g h", p=128),
                          out_sb[:])

    nc.compile()
    return nc


def _get_nc(nsup):
    if nsup not in _nc_cache:
        _nc_cache[nsup] = _build_nc(nsup)
    return _nc_cache[nsup]


def _indicator():
    # ind[ki, a, ko, m] = 1 iff m == 16a + ((ko*128 + ki) >> 4), m in [0, 64)
    ki = np.arange(128)[:, None, None, None]
    a = np.arange(4)[None, :, None, None]
    ko = np.arange(2)[None, None, :, None]
    m = np.arange(64)[None, None, None, :]
    ind = (m == 16 * a + ((ko * 128 + ki) >> 4))
    return np.ascontiguousarray(ind.astype(FP8NP))


def _make_in_maps(x, batch, W1, b1, W2, b2):
    # fp8 e4m3 input path: DMA bytes halve vs fp16 and DoubleRow matmuls
    # stream 2 fp8/partition/cycle; accumulation stays fp32 in PSUM.
    x8 = np.empty((N + 1, H), dtype=FP8NP)
    x8[:N] = np.asarray(x, dtype=np.float32)
    x8[N] = 0  # pad-row source
    batch_i = np.asarray(batch).astype(np.int64)
    W1 = np.ascontiguousarray(np.asarray(W1, dtype=np.float32))
    b1 = np.ascontiguousarray(np.asarray(b1, dtype=np.float32))
    W2 = np.ascontiguousarray(np.asarray(W2, dtype=np.float32))
    b2 = np.ascontiguousarray(np.asarray(b2, dtype=np.float32))

    cnt = np.bincount(batch_i, minlength=S).astype(np.int64)     # [S]
    seg_start = np.concatenate([[0], np.cumsum(cnt)])            # [S+1]
    rcp_all = (1.0 / np.maximum(cnt, 1.0)).astype(np.float32)

    # per-core block layout: segment s -> ceil(cnt/16) 16-row blocks
    nb = -(-cnt // GROWS)                                        # [S]
    bb = np.concatenate([[0], np.cumsum(nb.reshape(NCORES, SEG_PC), axis=1)
                         .reshape(-1)])  # global prefix is NOT what we want
    # per-core prefixes
    nb_c = nb.reshape(NCORES, SEG_PC)
    bb_c = np.zeros((NCORES, SEG_PC + 1), dtype=np.int64)
    bb_c[:, 1:] = np.cumsum(nb_c, axis=1)
    nblocks = bb_c[:, -1]                                        # [NCORES]
    # capacity: q-tile 4j+g holds window g's blocks [64j, 64j+64)
    wblocks = bb_c[:, 128::128] - bb_c[:, 0:-1:128]     # [NCORES, 4]
    nsup = int(-(-wblocks.max() // 64))
    cap_blocks = nsup * CH_ROWS // GROWS

    in_maps = []
    ind = _indicator()
    for k in range(NCORES):
        bbk = bb_c[k]
        # logical block -> local segment
        s_of_blk = np.searchsorted(bbk, np.arange(int(nblocks[k])),
                                   side="right") - 1
        # device block 64*(4j+g)+b -> logical block bb[128g] + 64j + b
        dev_blk = np.arange(cap_blocks)
        qq = dev_blk >> 6
        g_of = qq & 3
        lb = bbk[128 * g_of] + 64 * (qq >> 2) + (dev_blk & 63)
        valid = lb < bbk[128 * (g_of + 1)]
        lb_c = np.where(valid, lb, 0).astype(np.int64)

        # source row for each device row slot
        r_dev = np.arange(nsup * CH_ROWS, dtype=np.int64)
        blk = r_dev >> 4
        pos = r_dev & 15
        s_loc = s_of_blk[lb_c[blk]]
        row_in_seg = GROWS * (lb_c[blk] - bbk[s_loc]) + pos
        s_glob = SEG_PC * k + s_loc
        src = seg_start[s_glob] + row_in_seg
        src = np.where(valid[blk] & (row_in_seg < cnt[s_glob]), src, N)

        # device layout: r_dev = c*4096 + tl*256 + ko*128 + ki
        #   -> dram [c, ki, tl, ko, n]
        xs = x8[src].reshape(nsup, CH_MMS, 2, 128, H)
        xs = np.ascontiguousarray(xs.transpose(0, 3, 1, 2, 4)
                                  ).reshape(nsup * 128, CH_MMS * 512)

        # bbg[b, g, j] = seg_local(block 64*(4j+g)+b) - 128g, pads -> 1e4
        blkseg = np.where(valid, (s_of_blk[lb_c] - 128.0 * g_of), 1e4)
        bbg = np.ascontiguousarray(
            blkseg.reshape(nsup, 4, 64).transpose(2, 1, 0)).astype(np.float16)

        rcp = np.ascontiguousarray(
            rcp_all[SEG_PC * k:SEG_PC * (k + 1)].reshape(4, 128).T)
        in_maps.append({
            "xs": xs,
            "ind": ind,
            "bbg": bbg,
            "rcp": rcp,
            "w1": W1, "b1": b1, "w2": W2, "b2": b2,
        })
    return in_maps, nsup


def _run(x, batch, W1, b1, W2, b2, trace=False, **spmd_kwargs):
    in_maps, nsup = _make_in_maps(x, batch, W1, b1, W2, b2)
    nc = _get_nc(nsup)
    res = bass_utils.run_bass_kernel_spmd(
        nc, in_maps, core_ids=list(range(NCORES)), trace=trace, **spmd_kwargs)
    out = np.concatenate([res.results[k]["out"] for k in range(NCORES)], axis=0)
    return out.astype(np.float32, copy=False), res


def kernel(x, edge_index, edge_type, batch, W1, b1, W2, b2):
    out, _ = _run(x, batch, W1, b1, W2, b2)
    return out


# revision 28
# speedup vs baseline: 1.0262x; 1.0262x over previous
"""Trainium2 Bass kernel for nn_SimpleMLP (segment-mean + 2-layer MLP).

reference:
  sums = segment_sum(x, batch, 4096); cnt = segment_sum(ones, batch, 4096)
  pooled = sums / max(cnt, 1);  out = gelu(pooled @ W1 + b1) @ W2 + b2

v2 design (two-phase block reduction; replaces the per-row one-hot design
that was DVE-bound at ~200us):

  Distribution: `batch` is sorted; core k owns segments [512k, 512k+512).
  The host gathers each core's rows into a packed slab where every segment
  is padded to a whole number of 16-row blocks (pad rows read a zeros row,
  ~3% extra bytes), so each block is segment-pure. x is converted to
  fp8-e4m3 on the host (DMA bytes halve vs fp16; measured output rel err
  stays ~1e-2 < 2e-2 gate).

  Phase 1: fp8 DoubleRow matmuls (Ki=128 x Ko=2 = 256 rows each) against a
  CONSTANT 32-column block indicator produce 16-row block sums directly in
  PSUM ([128 blocks, 256] tiles, 4 col-group slices x 2 indicator phases).
  The moving operand is x, streamed at 2 fp8/partition/cycle (~546 GB/s >
  DMA rate), and LDWEIGHTS is only 64 columns (M=32) so it hides under the
  matmul. The Scalar engine evicts each PSUM tile to SBUF as fp16.

  Phase 2: a small block->segment one-hot (built on DVE from a host
  provided block->segment map, 16x fewer elements than a per-row one-hot)
  is matmul'd with the fp16 block sums, accumulating [128 segs, 256] per
  128-segment window. Host-computed 1/max(cnt,1) turns sums into means.

  Phase 3: replicated tiny MLP (fp32 matmuls, hardware Gelu) on the core's
  512 segments; host concatenates the 8 [512, 256] outputs.
"""
import sys

sys.path.insert(0, "/opt/trn_rl_repo")

from contextlib import ExitStack

import ml_dtypes
import numpy as np

import concourse.bacc as bacc
import concourse.mybir as mybir
import concourse.tile as tile
from concourse import bass_utils

F32 = mybir.dt.float32
F16 = mybir.dt.float16
F8 = mybir.dt.float8e4
FP8NP = ml_dtypes.float8_e4m3

N = 1048576
H = 256
S = 4096
NCORES = 8
SEG_PC = S // NCORES          # 512 segments per core
GROWS = 16                    # rows per block (segment padding granule)
MM_ROWS = 256                 # rows per DoubleRow matmul (Ki=128 * Ko=2)
CH_MMS = 16                   # matmuls per DMA chunk
CH_ROWS = CH_MMS * MM_ROWS    # 4096 rows per 1MB chunk
KQ = 8                        # q-tiles per one-hot DVE instruction

_nc_cache = {}


def _build_nc(nsup, use_gelu=True):
    nq = 4 * nsup             # [64,256] psum tiles (64 blocks each)
    nc = bacc.Bacc("TRN2", target_bir_lowering=False, debug=False,
                   num_devices=NCORES)
    xs_d = nc.dram_tensor("xs", [nsup * 128, CH_MMS * 512], F8,
                          kind="ExternalInput")
    ind_d = nc.dram_tensor("ind", [128, 4, 2, 64], F8, kind="ExternalInput")
    # block->segment map: q-tile 4j+g holds window g's blocks [64j, 64j+64);
    # bbg[b, g, j] = seg_local - 128g of block 64j+b of window g (fp16)
    bbg_d = nc.dram_tensor("bbg", [64, 4, nsup], F16, kind="ExternalInput")
    rcp_d = nc.dram_tensor("rcp", [128, 4], F32, kind="ExternalInput")
    w1_d = nc.dram_tensor("w1", [H, H], F32, kind="ExternalInput")
    b1_d = nc.dram_tensor("b1", [H], F32, kind="ExternalInput")
    w2_d = nc.dram_tensor("w2", [H, H], F32, kind="ExternalInput")
    b2_d = nc.dram_tensor("b2", [H], F32, kind="ExternalInput")
    out_d = nc.dram_tensor("out", [SEG_PC, H], F32, kind="ExternalOutput")

    DR = mybir.MatmulPerfMode.DoubleRow

    with tile.TileContext(nc) as tc, ExitStack() as ctx:
        const = ctx.enter_context(tc.tile_pool(name="const", bufs=1))
        xp = ctx.enter_context(tc.tile_pool(name="xp", bufs=6))
        ohp = ctx.enter_context(tc.tile_pool(name="ohp", bufs=3))
        psw = ctx.enter_context(tc.tile_pool(name="psw", bufs=2, space="PSUM"))
        psx = ctx.enter_context(tc.tile_pool(name="psx", bufs=4, space="PSUM"))
        sb = ctx.enter_context(tc.tile_pool(name="sb", bufs=1))

        # --- constants ---
        # dense per-position iota (values 0..127 repeated over KQ slots) for
        # the block one-hot compare; dense operand rides port 0
        iota_b = const.tile([128, KQ, 128], F16, name="iota_b")
        nc.gpsimd.iota(iota_b[:], pattern=[[0, KQ], [1, 128]], base=0,
                       channel_multiplier=0,
                       allow_small_or_imprecise_dtypes=True)
        pidx = const.tile([128, 1], F32)          # partition index
        nc.gpsimd.iota(pidx[:], pattern=[[0, 1]], base=0, channel_multiplier=1,
                       allow_small_or_imprecise_dtypes=True)
        identcmp = const.tile([128, 128], F32)
        nc.gpsimd.iota(identcmp[:], pattern=[[1, 128]], base=0,
                       channel_multiplier=0,
                       allow_small_or_imprecise_dtypes=True)
        ident = const.tile([128, 128], F32)       # identity for PE transpose
        nc.vector.tensor_scalar(ident[:], identcmp[:], pidx[:], None,
                                op0=mybir.AluOpType.is_equal)

        # --- weights / biases / maps ---
        ind_sb = const.tile([128, 4, 2, 64], F8)
        nc.gpsimd.dma_start(ind_sb[:], ind_d.ap())
        bbg_sb = const.tile([64, 4, nsup], F16)
        nc.gpsimd.dma_start(bbg_sb[:], bbg_d.ap())
        rcp_sb = const.tile([128, 4], F32)
        nc.gpsimd.dma_start(rcp_sb[:], rcp_d.ap())
        w1_sb = const.tile([128, 2, H], F32)
        nc.gpsimd.dma_start(w1_sb[:], w1_d.ap().rearrange("(k p) h -> p k h", p=128))
        w2_sb = const.tile([128, 2, H], F32)
        nc.gpsimd.dma_start(w2_sb[:], w2_d.ap().rearrange("(k p) h -> p k h", p=128))
        b1_sb = const.tile([128, 2], F32)
        nc.gpsimd.dma_start(b1_sb[:], b1_d.ap().rearrange("(m p) -> p m", p=128))
        b2_sb = const.tile([128, 2], F32)
        nc.gpsimd.dma_start(b2_sb[:], b2_d.ap().rearrange("(m p) -> p m", p=128))

        # fp16 16-row block sums, [64 blocks, 256] per q-tile (DoubleRow
        # matmuls must write PSUM at base partition 0, so everything lives
        # on partitions 0..63)
        bsums = const.tile([64, nq, H], F16, name="bsums")

        # --- phase 2 one-hots, pre-generated (depend only on constants) ---
        ohs = {}
        for g in range(4):
            for js in range(0, nsup, KQ):
                je = min(js + KQ, nsup)
                oh = const.tile([64, KQ, 128], F16, name=f"oh_{g}_{js}")
                bcast = (bbg_sb[:, g, js:je]
                         .rearrange("p (q u) -> p q u", u=1)
                         .broadcast_to((64, je - js, 128)))
                nc.vector.tensor_tensor(oh[:, :je - js, :],
                                        iota_b[0:64, :je - js, :], bcast,
                                        op=mybir.AluOpType.is_equal)
                ohs[g, js] = oh

        pooled = sb.tile([128, 4, H], F32)  # window g -> pooled[:, g, :]
        # one live PSUM accumulator per window, fed incrementally as q-tiles
        # are evicted (phase 2 rides the PE slack of the DMA-paced phase 1)
        pg = [psx.tile([128, H], F32, name=f"pg{g}", bufs=1) for g in range(4)]

        def emit_p2(c):
            for u in range(4):
                nc.tensor.matmul(pg[u][:], ohs[u, (c // KQ) * KQ][:, c % KQ, :],
                                 bsums[:, 4 * c + u, :],
                                 start=(c == 0), stop=(c == nsup - 1))

        # --- phase 1: block sums via constant-stationary DoubleRow ---
        hm = CH_MMS // 2
        for c in range(nsup):
            x_sb = xp.tile([128, CH_MMS, 2, H], F8)
            # split each 1MB chunk across the two HWDGE queues so they run
            # concurrently and release consumers at half-chunk granularity
            src = xs_d.ap()[c * 128:(c + 1) * 128, :] \
                .rearrange("p (t k n) -> p t k n", t=CH_MMS, k=2)
            nc.sync.dma_start(x_sb[:, 0:hm, :, :], src[:, 0:hm, :, :])
            nc.scalar.dma_start(x_sb[:, hm:CH_MMS, :, :],
                                src[:, hm:CH_MMS, :, :])
            for u in range(4):          # psum tiles per chunk (4 MMs each)
                pq = psw.tile([64, H], F32, name="pq", tag="pq")
                for a in range(4):      # indicator phase
                    tl = 4 * u + a
                    nc.tensor.matmul(pq[:],
                                     ind_sb[:, a, :, :],
                                     x_sb[:, tl, :, :],
                                     start=(a == 0), stop=(a == 3),
                                     perf_mode=DR)
                q = 4 * c + u
                # split PSUM evictions between ACT and DVE so neither
                # stalls its DMA issue stream
                if u % 2 == 0:
                    nc.scalar.activation(bsums[:, q, :], pq[:],
                                         mybir.ActivationFunctionType.Copy)
                else:
                    nc.vector.tensor_copy(bsums[:, q, :], pq[:])
            # phase-2 accumulation for the PREVIOUS chunk's q-tiles (their
            # evictions are long done, so these never stall the PE queue)
            if c >= 1:
                emit_p2(c - 1)
        emit_p2(nsup - 1)
        for g in range(4):
            nc.vector.tensor_scalar_mul(pooled[:, g, :], pg[g][:],
                                        rcp_sb[:, g:g + 1])

        # --- transpose pooled -> pooledT [128, 2, 512] (h-chunk, seg) ---
        pooledT = sb.tile([128, 2, SEG_PC], F32)
        for g in range(4):
            for j in range(2):
                pt = psx.tile([128, 128], F32, tag="pt", bufs=1)
                nc.tensor.transpose(pt[:], pooled[:, g, j * 128:(j + 1) * 128],
                                    ident[:])
                nc.vector.tensor_copy(pooledT[:, j, g * 128:(g + 1) * 128], pt[:])

        # --- MLP layer 1: hT = gelu(W1.T @ pooledT + b1) ---
        hT = sb.tile([128, 2, SEG_PC], F32)
        for m in range(2):
            ph = psx.tile([128, SEG_PC], F32, tag="ph", bufs=1)
            for k in range(2):
                nc.tensor.matmul(ph[:], w1_sb[:, k, m * 128:(m + 1) * 128],
                                 pooledT[:, k, :], start=(k == 0), stop=(k == 1))
            act = (mybir.ActivationFunctionType.Gelu if use_gelu
                   else mybir.ActivationFunctionType.Identity)
            nc.scalar.activation(hT[:, m, :], ph[:], act,
                                 bias=b1_sb[:, m:m + 1], scale=1.0)

        # --- MLP layer 2: oT = W2.T @ hT + b2 ---
        oT = sb.tile([128, 2, SEG_PC], F32)
        for m in range(2):
            ph = psx.tile([128, SEG_PC], F32, tag="ph", bufs=1)
            for k in range(2):
                nc.tensor.matmul(ph[:], w2_sb[:, k, m * 128:(m + 1) * 128],
                                 hT[:, k, :], start=(k == 0), stop=(k == 1))
            nc.scalar.activation(oT[:, m, :], ph[:],
                                 mybir.ActivationFunctionType.Identity,
                                 bias=b2_sb[:, m:m + 1], scale=1.0)

        # --- transpose back and store ---
        out_sb = sb.tile([128, 4, H], F32)
        for g in range(4):
            for j in range(2):
                pt = psx.tile([128, 128], F32, tag="pt", bufs=1)
                nc.tensor.transpose(pt[:], oT[:, j, g * 128:(g + 1) * 128],
                                    ident[:])
                nc.vector.tensor_copy(out_sb[:, g, j * 128:(j + 1) * 128], pt[:])
        nc.sync.dma_start(out_d.ap().rearrange("(g p) h -> p g h", p=128),
                          out_sb[:])

    nc.compile()
    return nc


def _get_nc(nsup):
    if nsup not in _nc_cache:
        _nc_cache[nsup] = _build_nc(nsup)
    return _nc_cache[nsup]


def _indicator():
    # ind[ki, a, ko, m] = 1 iff m == 16a + ((ko*128 + ki) >> 4), m in [0, 64)
    ki = np.arange(128)[:, None, None, None]
    a = np.arange(4)[None, :, None, None]
    ko = np.arange(2)[None, None, :, None]
    m = np.arange(64)[None, None, None, :]
    ind = (m == 16 * a + ((ko * 128 + ki) >> 4))
    return np.ascontiguousarray(ind.astype(FP8NP))


def _make_in_maps(x, batch, W1, b1, W2, b2):
    # fp8 e4m3 input path: DMA bytes halve vs fp16 and DoubleRow matmuls
    # stream 2 fp8/partition/cycle; accumulation stays fp32 in PSUM.
    x8 = np.empty((N + 1, H), dtype=FP8NP)
    x8[:N] = np.asarray(x, dtype=np.float32)
    x8[N] = 0  # pad-row source
    batch_i = np.asarray(batch).astype(np.int64)
    W1 = np.ascontiguousarray(np.asarray(W1, dtype=np.float32))
    b1 = np.ascontiguousarray(np.asarray(b1, dtype=np.float32))
    W2 = np.ascontiguousarray(np.asarray(W2, dtype=np.float32))
    b2 = np.ascontiguousarray(np.asarray(b2, dtype=np.float32))

    cnt = np.bincount(batch_i, minlength=S).astype(np.int64)     # [S]
    seg_start = np.concatenate([[0], np.cumsum(cnt)])            # [S+1]
    rcp_all = (1.0 / np.maximum(cnt, 1.0)).astype(np.float32)

    # per-core block layout: segment s -> ceil(cnt/16) 16-row blocks
    nb = -(-cnt // GROWS)                                        # [S]
    bb = np.concatenate([[0], np.cumsum(nb.reshape(NCORES, SEG_PC), axis=1)
                         .reshape(-1)])  # global prefix is NOT what we want
    # per-core prefixes
    nb_c = nb.reshape(NCORES, SEG_PC)
    bb_c = np.zeros((NCORES, SEG_PC + 1), dtype=np.int64)
    bb_c[:, 1:] = np.cumsum(nb_c, axis=1)
    nblocks = bb_c[:, -1]                                        # [NCORES]
    # capacity: q-tile 4j+g holds window g's blocks [64j, 64j+64)
    wblocks = bb_c[:, 128::128] - bb_c[:, 0:-1:128]     # [NCORES, 4]
    nsup = int(-(-wblocks.max() // 64))
    cap_blocks = nsup * CH_ROWS // GROWS

    in_maps = []
    ind = _indicator()
    for k in range(NCORES):
        bbk = bb_c[k]
        # logical block -> local segment
        s_of_blk = np.searchsorted(bbk, np.arange(int(nblocks[k])),
                                   side="right") - 1
        # device block 64*(4j+g)+b -> logical block bb[128g] + 64j + b
        dev_blk = np.arange(cap_blocks)
        qq = dev_blk >> 6
        g_of = qq & 3
        lb = bbk[128 * g_of] + 64 * (qq >> 2) + (dev_blk & 63)
        valid = lb < bbk[128 * (g_of + 1)]
        lb_c = np.where(valid, lb, 0).astype(np.int64)

        # source row for each device row slot
        r_dev = np.arange(nsup * CH_ROWS, dtype=np.int64)
        blk = r_dev >> 4
        pos = r_dev & 15
        s_loc = s_of_blk[lb_c[blk]]
        row_in_seg = GROWS * (lb_c[blk] - bbk[s_loc]) + pos
        s_glob = SEG_PC * k + s_loc
        src = seg_start[s_glob] + row_in_seg
        src = np.where(valid[blk] & (row_in_seg < cnt[s_glob]), src, N)

        # device layout: r_dev = c*4096 + tl*256 + ko*128 + ki
        #   -> dram [c, ki, tl, ko, n]
        xs = x8[src].reshape(nsup, CH_MMS, 2, 128, H)
        xs = np.ascontiguousarray(xs.transpose(0, 3, 1, 2, 4)
                                  ).reshape(nsup * 128, CH_MMS * 512)

        # bbg[b, g, j] = seg_local(block 64*(4j+g)+b) - 128g, pads -> 1e4
        blkseg = np.where(valid, (s_of_blk[lb_c] - 128.0 * g_of), 1e4)
        bbg = np.ascontiguousarray(
            blkseg.reshape(nsup, 4, 64).transpose(2, 1, 0)).astype(np.float16)

        rcp = np.ascontiguousarray(
            rcp_all[SEG_PC * k:SEG_PC * (k + 1)].reshape(4, 128).T)
        in_maps.append({
            "xs": xs,
            "ind": ind,
            "bbg": bbg,
            "rcp": rcp,
            "w1": W1, "b1": b1, "w2": W2, "b2": b2,
        })
    return in_maps, nsup


def _run(x, batch, W1, b1, W2, b2, trace=False, **spmd_kwargs):
    in_maps, nsup = _make_in_maps(x, batch, W1, b1, W2, b2)
    nc = _get_nc(nsup)
    res = bass_utils.run_bass_kernel_spmd(
        nc, in_maps, core_ids=list(range(NCORES)), trace=trace, **spmd_kwargs)
    out = np.concatenate([res.results[k]["out"] for k in range(NCORES)], axis=0)
    return out.astype(np.float32, copy=False), res


def kernel(x, edge_index, edge_type, batch, W1, b1, W2, b2):
    out, _ = _run(x, batch, W1, b1, W2, b2)
    return out


# revision 30
# speedup vs baseline: 1.2069x; 1.1761x over previous
"""Trainium2 Bass kernel for nn_SimpleMLP (segment-mean + 2-layer MLP).

reference:
  sums = segment_sum(x, batch, 4096); cnt = segment_sum(ones, batch, 4096)
  pooled = sums / max(cnt, 1);  out = gelu(pooled @ W1 + b1) @ W2 + b2

v2 design (two-phase block reduction; replaces the per-row one-hot design
that was DVE-bound at ~200us):

  Distribution: `batch` is sorted; core k owns segments [512k, 512k+512).
  The host gathers each core's rows into a packed slab where every segment
  is padded to a whole number of 16-row blocks (pad rows read a zeros row,
  ~3% extra bytes), so each block is segment-pure. x is converted to
  fp8-e4m3 on the host (DMA bytes halve vs fp16; measured output rel err
  stays ~1e-2 < 2e-2 gate).

  Phase 1: fp8 DoubleRow matmuls (Ki=128 x Ko=2 = 256 rows each) against a
  CONSTANT 32-column block indicator produce 16-row block sums directly in
  PSUM ([128 blocks, 256] tiles, 4 col-group slices x 2 indicator phases).
  The moving operand is x, streamed at 2 fp8/partition/cycle (~546 GB/s >
  DMA rate), and LDWEIGHTS is only 64 columns (M=32) so it hides under the
  matmul. The Scalar engine evicts each PSUM tile to SBUF as fp16.

  Phase 2: a small block->segment one-hot (built on DVE from a host
  provided block->segment map, 16x fewer elements than a per-row one-hot)
  is matmul'd with the fp16 block sums, accumulating [128 segs, 256] per
  128-segment window. Host-computed 1/max(cnt,1) turns sums into means.

  Phase 3: replicated tiny MLP (fp32 matmuls, hardware Gelu) on the core's
  512 segments; host concatenates the 8 [512, 256] outputs.
"""
import sys

sys.path.insert(0, "/opt/trn_rl_repo")

from contextlib import ExitStack

import ml_dtypes
import numpy as np

import concourse.bacc as bacc
import concourse.mybir as mybir
import concourse.tile as tile
from concourse import bass_utils

F32 = mybir.dt.float32
F16 = mybir.dt.float16
F8 = mybir.dt.float8e4
FP8NP = ml_dtypes.float8_e4m3

N = 1048576
H = 256
S = 4096
NCORES = 8
SEG_PC = S // NCORES          # 512 segments per core
GROWS = 16                    # rows per block (segment padding granule)
MM_ROWS = 256                 # rows per DoubleRow matmul (Ki=128 * Ko=2)
CH_MMS = 16                   # matmuls per DMA chunk
CH_ROWS = CH_MMS * MM_ROWS    # 4096 rows per 1MB chunk
KQ = 4                        # q-tiles per one-hot DVE instruction

_nc_cache = {}


def _build_nc(nsup, use_gelu=True):
    nq = 4 * nsup             # [64,256] psum tiles (64 blocks each)
    nc = bacc.Bacc("TRN2", target_bir_lowering=False, debug=False,
                   num_devices=NCORES)
    xs_d = nc.dram_tensor("xs", [nsup * 128, CH_MMS * 512], F8,
                          kind="ExternalInput")
    ind_d = nc.dram_tensor("ind", [128, 4, 2, 64], F8, kind="ExternalInput")
    # block->segment map: q-tile 4j+g holds window g's blocks [64j, 64j+64);
    # bbg[b, g, j] = seg_local - 128g of block 64j+b of window g (fp16)
    bbg_d = nc.dram_tensor("bbg", [64, 4, nsup], F16, kind="ExternalInput")
    rcp_d = nc.dram_tensor("rcp", [128, 4], F32, kind="ExternalInput")
    w1_d = nc.dram_tensor("w1", [H, H], F32, kind="ExternalInput")
    b1_d = nc.dram_tensor("b1", [H], F32, kind="ExternalInput")
    w2_d = nc.dram_tensor("w2", [H, H], F32, kind="ExternalInput")
    b2_d = nc.dram_tensor("b2", [H], F32, kind="ExternalInput")
    out_d = nc.dram_tensor("out", [SEG_PC, H], F32, kind="ExternalOutput")

    DR = mybir.MatmulPerfMode.DoubleRow

    with tile.TileContext(nc) as tc, ExitStack() as ctx:
        const = ctx.enter_context(tc.tile_pool(name="const", bufs=1))
        xp = ctx.enter_context(tc.tile_pool(name="xp", bufs=6))
        ohp = ctx.enter_context(tc.tile_pool(name="ohp", bufs=3))
        psw = ctx.enter_context(tc.tile_pool(name="psw", bufs=2, space="PSUM"))
        psx = ctx.enter_context(tc.tile_pool(name="psx", bufs=4, space="PSUM"))
        sb = ctx.enter_context(tc.tile_pool(name="sb", bufs=1))

        # --- constants ---
        # dense per-position iota (values 0..127 repeated over KQ slots) for
        # the block one-hot compare; dense operand rides port 0
        iota_b = const.tile([128, KQ, 128], F16, name="iota_b")
        nc.gpsimd.iota(iota_b[:], pattern=[[0, KQ], [1, 128]], base=0,
                       channel_multiplier=0,
                       allow_small_or_imprecise_dtypes=True)
        pidx = const.tile([128, 1], F32)          # partition index
        nc.gpsimd.iota(pidx[:], pattern=[[0, 1]], base=0, channel_multiplier=1,
                       allow_small_or_imprecise_dtypes=True)
        identcmp = const.tile([128, 128], F32)
        nc.gpsimd.iota(identcmp[:], pattern=[[1, 128]], base=0,
                       channel_multiplier=0,
                       allow_small_or_imprecise_dtypes=True)
        ident = const.tile([128, 128], F32)       # identity for PE transpose
        nc.vector.tensor_scalar(ident[:], identcmp[:], pidx[:], None,
                                op0=mybir.AluOpType.is_equal)

        # --- weights / biases / maps ---
        ind_sb = const.tile([128, 4, 2, 64], F8)
        nc.gpsimd.dma_start(ind_sb[:], ind_d.ap())
        bbg_sb = const.tile([64, 4, nsup], F16)
        nc.gpsimd.dma_start(bbg_sb[:], bbg_d.ap())
        rcp_sb = const.tile([128, 4], F32)
        nc.gpsimd.dma_start(rcp_sb[:], rcp_d.ap())
        w1_sb = const.tile([128, 2, H], F32)
        nc.gpsimd.dma_start(w1_sb[:], w1_d.ap().rearrange("(k p) h -> p k h", p=128))
        w2_sb = const.tile([128, 2, H], F32)
        nc.gpsimd.dma_start(w2_sb[:], w2_d.ap().rearrange("(k p) h -> p k h", p=128))
        b1_sb = const.tile([128, 2], F32)
        nc.gpsimd.dma_start(b1_sb[:], b1_d.ap().rearrange("(m p) -> p m", p=128))
        b2_sb = const.tile([128, 2], F32)
        nc.gpsimd.dma_start(b2_sb[:], b2_d.ap().rearrange("(m p) -> p m", p=128))

        # fp16 16-row block sums, [64 blocks, 256] per q-tile (DoubleRow
        # matmuls must write PSUM at base partition 0, so everything lives
        # on partitions 0..63)
        bsums = const.tile([64, nq, H], F16, name="bsums")

        # --- phase 2 one-hot machinery: generated just-in-time, two chunks
        # ahead of first use, so the DVE burst never stalls the PE queue ---
        ohs = {}

        def gen_oh(js):
            je = min(js + KQ, nsup)
            for g in range(4):
                oh = const.tile([64, KQ, 128], F16, name=f"oh_{g}_{js}")
                bcast = (bbg_sb[:, g, js:je]
                         .rearrange("p (q u) -> p q u", u=1)
                         .broadcast_to((64, je - js, 128)))
                nc.vector.tensor_tensor(oh[:, :je - js, :],
                                        iota_b[0:64, :je - js, :], bcast,
                                        op=mybir.AluOpType.is_equal)
                ohs[g, js] = oh

        pooled = sb.tile([128, 4, H], F32)  # window g -> pooled[:, g, :]
        # live PSUM accumulators, two windows per bank, fed incrementally as
        # q-tiles are evicted (phase 2 rides the PE slack of the DMA-paced
        # phase 1)
        pg01 = psx.tile([128, 2, H], F32, name="pg01", bufs=1)
        pg23 = psx.tile([128, 2, H], F32, name="pg23", bufs=1)
        pg = [pg01[:, 0, :], pg01[:, 1, :], pg23[:, 0, :], pg23[:, 1, :]]

        def emit_p2(c):
            for u in range(4):
                # start only on the first half of each shared PSUM bank: the
                # bank-wide zero region covers the partner window, whose
                # first write then accumulates onto pending-zero
                nc.tensor.matmul(pg[u], ohs[u, (c // KQ) * KQ][:, c % KQ, :],
                                 bsums[:, 4 * c + u, :],
                                 start=(c == 0 and u % 2 == 0),
                                 stop=(c == nsup - 1))

        gen_oh(0)

        # --- phase 1: block sums via constant-stationary DoubleRow ---
        hm = CH_MMS // 2
        for c in range(nsup):
            x_sb = xp.tile([128, CH_MMS, 2, H], F8)
            # split each 1MB chunk across the two HWDGE queues so they run
            # concurrently and release consumers at half-chunk granularity
            src = xs_d.ap()[c * 128:(c + 1) * 128, :] \
                .rearrange("p (t k n) -> p t k n", t=CH_MMS, k=2)
            nc.sync.dma_start(x_sb[:, 0:hm, :, :], src[:, 0:hm, :, :])
            nc.scalar.dma_start(x_sb[:, hm:CH_MMS, :, :],
                                src[:, hm:CH_MMS, :, :])
            for u in range(4):          # psum tiles per chunk (4 MMs each)
                pq = psw.tile([64, H], F32, name="pq", tag="pq")
                for a in range(4):      # indicator phase
                    tl = 4 * u + a
                    nc.tensor.matmul(pq[:],
                                     ind_sb[:, a, :, :],
                                     x_sb[:, tl, :, :],
                                     start=(a == 0), stop=(a == 3),
                                     perf_mode=DR)
                q = 4 * c + u
                # split PSUM evictions between ACT and DVE so neither
                # stalls its DMA issue stream
                if u % 2 == 0:
                    nc.scalar.activation(bsums[:, q, :], pq[:],
                                         mybir.ActivationFunctionType.Copy)
                else:
                    nc.vector.tensor_copy(bsums[:, q, :], pq[:])
            # stage upcoming one-hot groups two chunks before first use
            for js in range(KQ, nsup, KQ):
                if c == js - 2:
                    gen_oh(js)
            # phase-2 accumulation for the PREVIOUS chunk's q-tiles (their
            # evictions are long done, so these never stall the PE queue)
            if c >= 1:
                emit_p2(c - 1)
        emit_p2(nsup - 1)
        for g in range(4):
            nc.vector.tensor_scalar_mul(pooled[:, g, :], pg[g],
                                        rcp_sb[:, g:g + 1])

        # --- transpose pooled -> pooledT [128, 2, 512] (h-chunk, seg) ---
        pooledT = sb.tile([128, 2, SEG_PC], F32)
        for g in range(4):
            for j in range(2):
                pt = psx.tile([128, 128], F32, tag="pt", bufs=2)
                nc.tensor.transpose(pt[:], pooled[:, g, j * 128:(j + 1) * 128],
                                    ident[:])
                nc.vector.tensor_copy(pooledT[:, j, g * 128:(g + 1) * 128], pt[:])

        # --- MLP layer 1: hT = gelu(W1.T @ pooledT + b1) ---
        hT = sb.tile([128, 2, SEG_PC], F32)
        for m in range(2):
            ph = psx.tile([128, SEG_PC], F32, tag="ph", bufs=2)
            for k in range(2):
                nc.tensor.matmul(ph[:], w1_sb[:, k, m * 128:(m + 1) * 128],
                                 pooledT[:, k, :], start=(k == 0), stop=(k == 1))
            act = (mybir.ActivationFunctionType.Gelu if use_gelu
                   else mybir.ActivationFunctionType.Identity)
            nc.scalar.activation(hT[:, m, :], ph[:], act,
                                 bias=b1_sb[:, m:m + 1], scale=1.0)

        # --- MLP layer 2: oT = W2.T @ hT + b2 ---
        oT = sb.tile([128, 2, SEG_PC], F32)
        for m in range(2):
            ph = psx.tile([128, SEG_PC], F32, tag="ph", bufs=2)
            for k in range(2):
                nc.tensor.matmul(ph[:], w2_sb[:, k, m * 128:(m + 1) * 128],
                                 hT[:, k, :], start=(k == 0), stop=(k == 1))
            nc.scalar.activation(oT[:, m, :], ph[:],
                                 mybir.ActivationFunctionType.Identity,
                                 bias=b2_sb[:, m:m + 1], scale=1.0)

        # --- transpose back and store ---
        out_sb = sb.tile([128, 4, H], F32)
        for g in range(4):
            for j in range(2):
                pt = psx.tile([128, 128], F32, tag="pt", bufs=2)
                nc.tensor.transpose(pt[:], oT[:, j, g * 128:(g + 1) * 128],
                                    ident[:])
                nc.vector.tensor_copy(out_sb[:, g, j * 128:(j + 1) * 128], pt[:])
        nc.sync.dma_start(out_d.ap().rearrange("(g p) h -> p g h", p=128),
                          out_sb[:])

    nc.compile()
    return nc


def _get_nc(nsup):
    if nsup not in _nc_cache:
        _nc_cache[nsup] = _build_nc(nsup)
    return _nc_cache[nsup]


def _indicator():
    # ind[ki, a, ko, m] = 1 iff m == 16a + ((ko*128 + ki) >> 4), m in [0, 64)
    ki = np.arange(128)[:, None, None, None]
    a = np.arange(4)[None, :, None, None]
    ko = np.arange(2)[None, None, :, None]
    m = np.arange(64)[None, None, None, :]
    ind = (m == 16 * a + ((ko * 128 + ki) >> 4))
    return np.ascontiguousarray(ind.astype(FP8NP))


def _make_in_maps(x, batch, W1, b1, W2, b2):
    # fp8 e4m3 input path: DMA bytes halve vs fp16 and DoubleRow matmuls
    # stream 2 fp8/partition/cycle; accumulation stays fp32 in PSUM.
    x8 = np.empty((N + 1, H), dtype=FP8NP)
    x8[:N] = np.asarray(x, dtype=np.float32)
    x8[N] = 0  # pad-row source
    batch_i = np.asarray(batch).astype(np.int64)
    W1 = np.ascontiguousarray(np.asarray(W1, dtype=np.float32))
    b1 = np.ascontiguousarray(np.asarray(b1, dtype=np.float32))
    W2 = np.ascontiguousarray(np.asarray(W2, dtype=np.float32))
    b2 = np.ascontiguousarray(np.asarray(b2, dtype=np.float32))

    cnt = np.bincount(batch_i, minlength=S).astype(np.int64)     # [S]
    seg_start = np.concatenate([[0], np.cumsum(cnt)])            # [S+1]
    rcp_all = (1.0 / np.maximum(cnt, 1.0)).astype(np.float32)

    # per-core block layout: segment s -> ceil(cnt/16) 16-row blocks
    nb = -(-cnt // GROWS)                                        # [S]
    bb = np.concatenate([[0], np.cumsum(nb.reshape(NCORES, SEG_PC), axis=1)
                         .reshape(-1)])  # global prefix is NOT what we want
    # per-core prefixes
    nb_c = nb.reshape(NCORES, SEG_PC)
    bb_c = np.zeros((NCORES, SEG_PC + 1), dtype=np.int64)
    bb_c[:, 1:] = np.cumsum(nb_c, axis=1)
    nblocks = bb_c[:, -1]                                        # [NCORES]
    # capacity: q-tile 4j+g holds window g's blocks [64j, 64j+64)
    wblocks = bb_c[:, 128::128] - bb_c[:, 0:-1:128]     # [NCORES, 4]
    nsup = int(-(-wblocks.max() // 64))
    cap_blocks = nsup * CH_ROWS // GROWS

    in_maps = []
    ind = _indicator()
    for k in range(NCORES):
        bbk = bb_c[k]
        # logical block -> local segment
        s_of_blk = np.searchsorted(bbk, np.arange(int(nblocks[k])),
                                   side="right") - 1
        # device block 64*(4j+g)+b -> logical block bb[128g] + 64j + b
        dev_blk = np.arange(cap_blocks)
        qq = dev_blk >> 6
        g_of = qq & 3
        lb = bbk[128 * g_of] + 64 * (qq >> 2) + (dev_blk & 63)
        valid = lb < bbk[128 * (g_of + 1)]
        lb_c = np.where(valid, lb, 0).astype(np.int64)

        # source row for each device row slot
        r_dev = np.arange(nsup * CH_ROWS, dtype=np.int64)
        blk = r_dev >> 4
        pos = r_dev & 15
        s_loc = s_of_blk[lb_c[blk]]
        row_in_seg = GROWS * (lb_c[blk] - bbk[s_loc]) + pos
        s_glob = SEG_PC * k + s_loc
        src = seg_start[s_glob] + row_in_seg
        src = np.where(valid[blk] & (row_in_seg < cnt[s_glob]), src, N)

        # device layout: r_dev = c*4096 + tl*256 + ko*128 + ki
        #   -> dram [c, ki, tl, ko, n]
        xs = x8[src].reshape(nsup, CH_MMS, 2, 128, H)
        xs = np.ascontiguousarray(xs.transpose(0, 3, 1, 2, 4)
                                  ).reshape(nsup * 128, CH_MMS * 512)

        # bbg[b, g, j] = seg_local(block 64*(4j+g)+b) - 128g, pads -> 1e4
        blkseg = np.where(valid, (s_of_blk[lb_c] - 128.0 * g_of), 1e4)
        bbg = np.ascontiguousarray(
            blkseg.reshape(nsup, 4, 64).transpose(2, 1, 0)).astype(np.float16)

        rcp = np.ascontiguousarray(
            rcp_all[SEG_PC * k:SEG_PC * (k + 1)].reshape(4, 128).T)
        in_maps.append({
            "xs": xs,
            "ind": ind,
            "bbg": bbg,
            "rcp": rcp,
            "w1": W1, "b1": b1, "w2": W2, "b2": b2,
        })
    return in_maps, nsup


def _run(x, batch, W1, b1, W2, b2, trace=False, **spmd_kwargs):
    in_maps, nsup = _make_in_maps(x, batch, W1, b1, W2, b2)
    nc = _get_nc(nsup)
    res = bass_utils.run_bass_kernel_spmd(
        nc, in_maps, core_ids=list(range(NCORES)), trace=trace, **spmd_kwargs)
    out = np.concatenate([res.results[k]["out"] for k in range(NCORES)], axis=0)
    return out.astype(np.float32, copy=False), res


def kernel(x, edge_index, edge_type, batch, W1, b1, W2, b2):
    out, _ = _run(x, batch, W1, b1, W2, b2)
    return out
